# revision 32
# baseline (speedup 1.0000x reference)
"""RWKV time-mixing (C=4096) on 8 trn2 NeuronCores.

Strategy (tensor-parallel over channels, M=8 cores, S=C/M=512):
  - Core c owns channels sl = [c*512, (c+1)*512).
  - HBM-bound: streams the four weight matrices once in bf16 (16 MB/core;
    rel-err 3.3e-3 vs the 2e-2 budget; fp8 measured over budget). A DMA-only
    probe measured the per-core stream limit at ~425 B/ns on ONE sync-HWDGE
    ring; a dual-ring probe split the same 425 between rings — HBM-capped,
    so all weights ride the single sync ring.
  - Stream order wv, kw, rw, ow: vv lands first, the whole WKV recurrence +
    state update (kk-stage, ~3 us of vector/scalar work) hides under the rw
    stream, and after rw only the short r=sigmoid(rr) -> y=r*wkv -> bf16
    chain (~1.3 us) gates phase 2 — which then chases the arriving ow
    chunks with no other dependency.
  - Every chunk gets a dedicated SBUF buffer (16 MB resident, no reuse
    waits — a reuse stall cost the old kernel ~0.7 us mid-stream).
  - Phase 1: kk/vv/rr shards via host-transposed weights; contraction on
    SBUF partitions, x-vector column stationary, weight tiles moving at
    N=512. The [1,512]->[128,4] transpose of each result runs on the PE
    (zero-padded stationary + e0 unit vector, fp32r).
  - Phase 2: partial = ow[:, sl] @ y as 4 k-tiles x 8 n-banks into a
    [1,4096] PSUM row. ow's last k-tile streams as a 512 KB half (banks
    0-3) plus two 256 KB quarters, so only two stop-matmuls + two parallel
    PSUM->SBUF copies + a 2 KB DMA follow the final weight byte. Copies
    alternate vector/scalar per bank; the output leaves in bf16 as three
    pieces on the sync ring as soon as their banks are staged. Host sums
    the 8 per-core partials in f32.
  - new_state = x exactly (token shift), assembled on host. nst (state
    a/b/p) leaves mid-kernel via gpsimd right after the kk-stage.
  - PE p-state: the PE stalls ~6-8 us after its FIRST matmul, then runs
    ~3 us at half clock. A tiny f32 matmul (dep: one vector memset) fires
    at ~6.3 us so the stall drains while the first chunks stream; pinned
    filler matmuls (deps: last rw chunk / ofull[0]) bridge the two ~1.2 us
    PE-idle windows at the rw->phase-2 transition, which otherwise de-ramp
    the PE to half clock for the first 8 phase-2 matmuls.
  - exec_time is measured from the framework's first MEMSET to the end of
    the (fixed, ~6.7 us) whole-semaphore-file teardown; head before the
    first MEMSET is free, everything after the last output byte counts.
    The last-byte -> consumer semaphore visibility latency is ~1.5 us of
    fixed NOC cost (the sem packet lands 6 ns after the data).
  - The device is bimodal run-to-run (~±10%): in the slow mode a
    power/thermal governor clamps DMA to ~340 B/ns after ~20 us of
    sustained streaming AND halves the PE clock mid-burst (observed with
    zero-gap matmul cadence), on pure-DMA probes too; a mid mode keeps the
    stream fast but stretches the DMA-sem visibility latency 1.5->3.2 us.
    Not controllable from the kernel; fast-mode ~57.7-58.9 us, slow-mode
    ~64-66 us. The post-stream tail is ~13.3 us, of which ~6.9 us is the
    fixed semaphore-file wipe (133 ns/sem, clock-invariant) and ~5.5 us is
    serial sem/NOC/descriptor latencies with no remaining mechanism to
    overlap.

k-index convention (phase 1): k = p*32 + n (p = partition, n = k-tile id),
so W.T.reshape(128, 32, 512) puts k-tile n at [:, n, :] with contraction on
partitions, and x.reshape(128, 32) column n is the matching stationary vec.
WKV-side layout is [128, 4] (channel = j*128 + p), matching the PE-reshape
output and phase-2's per-k-tile contraction over ow[:, sl].T rows.
"""

import ml_dtypes
import numpy as np

import concourse.bass as bass
import concourse.mybir as mybir
import concourse.tile as tile
from concourse import bacc, bass_utils

C = 4096
NCORES = 8
S = C // NCORES          # 512 channels per core
P = 128
KT = C // P              # 32 k-tiles in phase 1
OW_KT = S // P           # 4 k-tiles in phase 2
CHUNKS_FULL = [(0, 8), (8, 8), (16, 8), (24, 8)]

F32 = mybir.dt.float32
F32R = mybir.dt.float32r
BF16 = mybir.dt.bfloat16
AF = mybir.ActivationFunctionType
NP_BF16 = ml_dtypes.bfloat16

# xvecs layout: [128, 96] = xk[0:32] | xv[32:64] | xr[64:96] (bf16)
# svecs layout: [128, 20] = aa[0:4] | bb[4:8] | pp[8:12] | tf[12:16] | td[16:20]
XVECS_W = 3 * KT
SVECS_W = 5 * 4
OFF_XK, OFF_XV, OFF_XR = 0, KT, 2 * KT
OFF_AA, OFF_BB, OFF_PP, OFF_TF, OFF_TD = 0, 4, 8, 12, 16


def _build():
    nc = bacc.Bacc("TRN2", target_bir_lowering=False, debug=False,
                   num_devices=NCORES)

    xvecs_d = nc.dram_tensor("xvecs", [P, XVECS_W], BF16, kind="ExternalInput")
    e0_d = nc.dram_tensor("e0", [P, 2], F32R, kind="ExternalInput")
    svecs_d = nc.dram_tensor("svecs", [P, SVECS_W], F32, kind="ExternalInput")
    wr_d = nc.dram_tensor("wr", [P, KT, S], BF16, kind="ExternalInput")
    wk_d = nc.dram_tensor("wk", [P, KT, S], BF16, kind="ExternalInput")
    wv_d = nc.dram_tensor("wv", [P, KT, S], BF16, kind="ExternalInput")
    wo_d = nc.dram_tensor("wo", [OW_KT, P, C], BF16, kind="ExternalInput")

    # partial leaves in bf16: the per-core quantization (~0.23% rms, random
    # sign) adds ~2e-3 to the 2e-2 budget and halves the critical-path
    # output DMA
    partial_d = nc.dram_tensor("partial", [1, C], BF16, kind="ExternalOutput")
    nst_d = nc.dram_tensor("nst", [P, 12], F32, kind="ExternalOutput")

    with tile.TileContext(nc) as tc:
        with (
            tc.tile_pool(name="owh", bufs=1) as ohpool,
            tc.tile_pool(name="const", bufs=1) as const,
            tc.tile_pool(name="w", bufs=12) as wpool,
            tc.tile_pool(name="ow", bufs=3) as opool,
            tc.tile_pool(name="small", bufs=1) as small,
        ):
            # z2 feeds the PE p-state ramp-trigger matmul; its memset leads
            # the VECTOR queue (vector is otherwise idle until ~20 us) so
            # the trigger fires as early as possible.
            z2 = small.tile([P, 2], F32)
            nc.vector.memset(z2[:], 0.0)

            xvecs = const.tile([P, XVECS_W], BF16)
            e0 = const.tile([P, 2], F32R)
            svecs = const.tile([P, SVECS_W], F32)

            # small inputs on the gpsimd SWDGE ring (the sync ring carries
            # only 8 KB weight rows — a probe showed xvecs' 192 B rows wedge
            # it for ~0.5 us). xvecs first: it gates the first phase-1
            # matmul (~13 us, after the PE p-state stall); e0 is needed by
            # the first PE-reshape (~21 us), svecs by the kk-stage (~30 us).
            nc.gpsimd.dma_start(xvecs[:], xvecs_d[:])
            nc.gpsimd.dma_start(e0[:], e0_d[:])
            nc.gpsimd.dma_start(svecs[:], svecs_d[:])

            # preload the ACT exp LUT off the critical path (the only ACT
            # table the kernel uses: sigmoid is computed via exp+reciprocal)
            warm = small.tile([1, 4], F32)
            nc.gpsimd.memset(warm[:], 0.0)
            warm2 = small.tile([1, 4], F32)
            nc.scalar.activation(warm2[:], warm[:], AF.Exp)

            # stage for the PE reshape: partition 0 carries the phase-1
            # results, rows 1-127 must be finite (they multiply e0's zeros).
            # f32r memset is invalid ISA, so zero an f32 twin and cast-copy
            # (vector; off the critical path).
            stage_z = small.tile([P, 3 * S], F32)
            nc.vector.memset(stage_z[:], 0.0)
            stage = small.tile([P, 3 * S], F32R)
            nc.vector.tensor_copy(stage[:], stage_z[:])

            aa = svecs[:, OFF_AA:OFF_AA + 4]
            bb = svecs[:, OFF_BB:OFF_BB + 4]
            pp = svecs[:, OFF_PP:OFF_PP + 4]
            tf = svecs[:, OFF_TF:OFF_TF + 4]
            td = svecs[:, OFF_TD:OFF_TD + 4]

            def t4(name):
                return small.tile([P, 4], F32, name=name)

            nst = small.tile([P, 12], F32)
            na, nb_t, p2 = nst[:, 0:4], nst[:, 4:8], nst[:, 8:12]
            # rkv layout follows stream order: vv | kk | rr
            rkv = small.tile([P, 12], F32)
            vv = rkv[:, 0:4]
            kk = rkv[:, 4:8]
            rr128 = rkv[:, 8:12]

            wkv = t4("wkv")
            y = t4("y")
            y_r = small.tile([P, 4], BF16, name="y_r")

            # ---- phase 1 + overlapped recurrence ---------------------------
            wdrams = [wv_d, wk_d, wr_d]
            xoffs = [OFF_XV, OFF_XK, OFF_XR]

            with tc.tile_pool(name="ps1", bufs=1, space="PSUM") as ps1:
                psums = [ps1.tile([1, S], F32, name=f"ps_{i}") for i in range(3)]
                rs_ps = ps1.tile([P, 24], F32, name="rs_ps")
                fl_ps = ps1.tile([1, S], F32, name="fl_ps")

                # PE p-state ramp trigger (see module docstring)
                zps = ps1.tile([2, 2], F32, name="zps")
                nc.tensor.matmul(zps[:], lhsT=z2[:], rhs=z2[:], start=True, stop=True)

                last_rw_chunk = None
                for wi in range(3):
                    for ci, (kt0, nkt) in enumerate(CHUNKS_FULL):
                        wt = wpool.tile([P, nkt, S], BF16, tag="wchunk")
                        nc.sync.dma_start(
                            wt[:], wdrams[wi][:, kt0:kt0 + nkt, :])
                        if wi == 2 and ci == len(CHUNKS_FULL) - 1:
                            last_rw_chunk = wt
                        for tl in range(nkt):
                            kt = kt0 + tl
                            nc.tensor.matmul(
                                psums[wi][:],
                                lhsT=xvecs[:, xoffs[wi] + kt:xoffs[wi] + kt + 1],
                                rhs=wt[:, tl, :],
                                start=(kt == 0),
                                stop=(kt == KT - 1),
                            )

                    if wi == 2:
                        # keep the PE busy through the rr CAST window so it
                        # does not de-ramp before phase 2 (measured: an idle
                        # stretch over ~1.2 us halves the PE clock for the
                        # next ~3 us). Deps on the last rw chunk pin these
                        # AFTER the rw matmuls in the schedule.
                        for i in range(3):
                            nc.tensor.matmul(
                                fl_ps[:],
                                lhsT=xvecs[:, i:i + 1],
                                rhs=last_rw_chunk[:, 7, :],
                                start=True,
                                stop=True,
                            )

                    # matrix wi fully reduced: transpose its [1,512] row into
                    # the [128,4] WKV layout while the next matrix streams
                    if wi < 2:
                        nc.vector.tensor_copy(
                            stage[0:1, wi * S:(wi + 1) * S], psums[wi][:])
                    else:
                        nc.scalar.copy(stage[0:1, wi * S:(wi + 1) * S], psums[wi][:])
                    for j in range(OW_KT):
                        c2 = 2 * (wi * 4 + j)
                        nc.tensor.matmul(
                            rs_ps[:, c2:c2 + 2],
                            lhsT=stage[:, wi * S + j * P:wi * S + (j + 1) * P],
                            rhs=e0[:],
                            start=True,
                            stop=True,
                        )
                    nc.vector.tensor_copy(
                        rkv[:, wi * 4:wi * 4 + 4],
                        rs_ps[:, 2 * wi * 4:2 * wi * 4 + 8:2])

                    if wi == 1:
                        # kk-stage: vv and kk are both in hand — compute the
                        # ENTIRE WKV recurrence and state update here; it all
                        # hides under the rw stream.
                        ww1 = t4("ww1")
                        nc.vector.tensor_add(ww1, tf, kk)
                        p1 = t4("p1")
                        nc.vector.tensor_max(p1, pp, ww1)
                        d1 = t4("d1")
                        nc.vector.tensor_sub(d1, pp, p1)
                        e1a = t4("e1a")
                        nc.scalar.activation(e1a, d1, AF.Exp)
                        d2 = t4("d2")
                        nc.vector.tensor_sub(d2, ww1, p1)
                        e2a = t4("e2a")
                        nc.scalar.activation(e2a[:], d2, AF.Exp)
                        acc_a = t4("acc_a")
                        nc.vector.tensor_mul(acc_a[:], e1a, aa)
                        tmp_a = t4("tmp_a")
                        nc.vector.tensor_mul(tmp_a, e2a[:], vv)
                        nc.vector.tensor_add(acc_a[:], acc_a[:], tmp_a)
                        acc_b = t4("acc_b")
                        nc.vector.tensor_mul(acc_b, e1a, bb)
                        nc.vector.tensor_add(acc_b, acc_b, e2a[:])
                        binv = t4("binv")
                        nc.vector.reciprocal(binv[:], acc_b)
                        nc.vector.tensor_mul(wkv[:], acc_a[:], binv[:])
                        # state update (decayed accumulators) -> nst
                        ww2 = t4("ww2")
                        nc.vector.tensor_add(ww2, pp, td)
                        nc.vector.tensor_max(p2, ww2, kk)
                        d3 = t4("d3")
                        nc.vector.tensor_sub(d3, ww2, p2)
                        e1b = t4("e1b")
                        nc.scalar.activation(e1b, d3, AF.Exp)
                        d4 = t4("d4")
                        nc.vector.tensor_sub(d4, kk, p2)
                        e2b = t4("e2b")
                        nc.scalar.activation(e2b[:], d4, AF.Exp)
                        nc.vector.tensor_mul(na, e1b, aa)
                        tmp_b = t4("tmp_b")
                        nc.vector.tensor_mul(tmp_b, e2b[:], vv)
                        nc.vector.tensor_add(na, na, tmp_b)
                        nc.vector.tensor_mul(nb_t, e1b, bb)
                        nc.vector.tensor_add(nb_t, nb_t, e2b[:])
                        # state leaves mid-kernel, fully hidden
                        nc.gpsimd.dma_start(nst_d[:], nst[:])
                    elif wi == 2:
                        # minimal rr tail on the y-critical path:
                        # r = sigmoid(rr) = 1/(1+exp(-rr)), y = r*wkv
                        er = t4("er")
                        nc.scalar.activation(er[:], rr128, AF.Exp, scale=-1.0)
                        rp1 = t4("rp1")
                        nc.vector.tensor_scalar_add(rp1[:], er[:], 1.0)
                        r128 = t4("r128")
                        nc.vector.reciprocal(r128[:], rp1[:])
                        nc.vector.tensor_mul(y[:], r128[:], wkv[:])
                        nc.vector.tensor_copy(y_r[:], y[:])

                # ow weight DMAs: the in-order sync ring plays them right
                # after the phase-1 weight DMAs; by the time the first chunk
                # lands y_r is (nearly) ready and the phase-2 matmuls chase
                # the arriving chunks. k-tiles 0-2 stream as full 1 MB chunks
                # (8 KB rows); k-tile 3 as 2x512KB halves.
                ofull = []
                for tt in range(OW_KT - 1):
                    ot = opool.tile([P, C], BF16, tag="owfull")
                    nc.sync.dma_start(ot[:], wo_d[tt][:, :])
                    ofull.append(ot)
                # tt=3 streams last: banks 0-3 as one 512 KB half, banks 4-5
                # and 6-7 as 256 KB quarters, so the after-last-byte tail is
                # just two N=512 stop-matmuls + two parallel copies + a 4 KB
                # output DMA.
                opieces = []
                for (c0, c1) in ((0, 2048), (2048, 3072), (3072, 4096)):
                    ot = ohpool.tile([P, c1 - c0], BF16, tag=f"owp{c0}")
                    nc.sync.dma_start(ot[:], wo_d[OW_KT - 1][:, c0:c1])
                    opieces.append((c0, ot))

                # bridge the PE from the rr reshape to the first phase-2
                # matmul (the y_r vector chain takes ~1.5 us); deps on
                # ofull[0] pin these after the reshape in the schedule.
                for i in range(3):
                    nc.tensor.matmul(
                        fl_ps[:],
                        lhsT=xvecs[:, 3 + i:4 + i],
                        rhs=ofull[0][:, 0:S],
                        start=True,
                        stop=True,
                    )

            # ---- phase 2: partial = ow[:, sl] @ y ---------------------------
            with tc.tile_pool(name="ps2", bufs=1, space="PSUM") as ps2:
                ow_ps = ps2.tile([1, C], F32)
                out_sb = small.tile([1, C], BF16)
                for tt in range(OW_KT):
                    for nb in range(C // 512):
                        if tt < OW_KT - 1:
                            rhs = ofull[tt][:, nb * 512:(nb + 1) * 512]
                        else:
                            pi = 0 if nb < 4 else (1 if nb < 6 else 2)
                            c0, ot = opieces[pi]
                            rhs = ot[:, nb * 512 - c0:(nb + 1) * 512 - c0]
                        nc.tensor.matmul(
                            ow_ps[:, nb * 512:(nb + 1) * 512],
                            lhsT=y_r[:, tt:tt + 1],
                            rhs=rhs,
                            start=(tt == 0),
                            stop=(tt == OW_KT - 1),
                        )
                        if tt == OW_KT - 1:
                            # bank nb complete; copy out while later banks
                            # are still accumulating (gpsimd cannot read
                            # PSUM, so alternate vector/scalar)
                            sl_ = slice(nb * 512, (nb + 1) * 512)
                            if nb % 2 == 0:
                                nc.vector.tensor_copy(
                                    out_sb[:, sl_], ow_ps[:, sl_])
                            else:
                                nc.scalar.copy(out_sb[:, sl_], ow_ps[:, sl_])
                            if nb in (3, 5, 7):
                                # each output piece leaves as soon as its
                                # banks are staged; only the last 4 KB piece
                                # is on the critical path
                                lo = 0 if nb == 3 else (nb - 1) * 512
                                nc.sync.dma_start(
                                    partial_d[:, lo:(nb + 1) * 512],
                                    out_sb[:, lo:(nb + 1) * 512])

    nc.compile()
    return nc


def _prep_in_maps(x, state, state_a, state_b, state_p,
                  time_mix_k, time_mix_v, time_mix_r,
                  time_first, time_decay, kw, vw, rw, ow):
    f = lambda a: np.ascontiguousarray(np.asarray(a), dtype=np.float32)
    x, state = f(x), f(state)
    tmk, tmv, tmr = f(time_mix_k), f(time_mix_v), f(time_mix_r)
    xk = (x * tmk + state * (1.0 - tmk)).reshape(P, KT)
    xv = (x * tmv + state * (1.0 - tmv)).reshape(P, KT)
    xr = (x * tmr + state * (1.0 - tmr)).reshape(P, KT)
    aa, bb, pp = f(state_a), f(state_b), f(state_p)
    tf, td = f(time_first), f(time_decay)
    kw, vw, rw, ow = f(kw), f(vw), f(rw), f(ow)

    xvecs = np.zeros((P, XVECS_W), dtype=np.float32)
    xvecs[:, OFF_XK:OFF_XK + KT] = xk
    xvecs[:, OFF_XV:OFF_XV + KT] = xv
    xvecs[:, OFF_XR:OFF_XR + KT] = xr
    xvecs = xvecs.astype(NP_BF16)

    e0 = np.zeros((P, 2), dtype=np.float32)
    e0[0, 0] = 1.0

    wb = lambda a: np.ascontiguousarray(a).astype(NP_BF16)

    # WKV-side [128, 4] layout: channel = j*128 + p
    pm = lambda v: np.ascontiguousarray(v.reshape(OW_KT, P).T)
    in_maps = []
    for c in range(NCORES):
        sl = slice(c * S, (c + 1) * S)
        svecs = np.empty((P, SVECS_W), dtype=np.float32)
        svecs[:, OFF_AA:OFF_AA + 4] = pm(aa[sl])
        svecs[:, OFF_BB:OFF_BB + 4] = pm(bb[sl])
        svecs[:, OFF_PP:OFF_PP + 4] = pm(pp[sl])
        svecs[:, OFF_TF:OFF_TF + 4] = pm(tf[sl])
        svecs[:, OFF_TD:OFF_TD + 4] = pm(td[sl])
        in_maps.append({
            "xvecs": xvecs,
            "e0": e0,
            "svecs": svecs,
            "wr": wb(rw[sl, :].T).reshape(P, KT, S),
            "wk": wb(kw[sl, :].T).reshape(P, KT, S),
            "wv": wb(vw[sl, :].T).reshape(P, KT, S),
            "wo": wb(ow[:, sl].T).reshape(OW_KT, P, C),
        })
    return in_maps, x


_NC_CACHE = None


def _run(inputs, trace=False):
    global _NC_CACHE
    if _NC_CACHE is None:
        _NC_CACHE = _build()
    nc = _NC_CACHE
    in_maps, x = _prep_in_maps(**inputs)
    res = bass_utils.run_bass_kernel_spmd(
        nc, in_maps, core_ids=list(range(NCORES)), trace=trace)

    out = np.zeros(C, dtype=np.float32)
    new_a = np.empty(C, dtype=np.float32)
    new_b = np.empty(C, dtype=np.float32)
    new_p = np.empty(C, dtype=np.float32)
    for c in range(NCORES):
        r = res.results[c]
        out += np.asarray(r["partial"], dtype=np.float32).reshape(C)
        sl = slice(c * S, (c + 1) * S)
        nst = r["nst"]
        # [p, j] -> channel j*128 + p
        new_a[sl] = nst[:, 0:4].T.reshape(S)
        new_b[sl] = nst[:, 4:8].T.reshape(S)
        new_p[sl] = nst[:, 8:12].T.reshape(S)
    return (out, x.copy(), new_a, new_b, new_p), res


def kernel(**inputs):
    outs, _ = _run(inputs, trace=False)
    return outs


# revision 33
# speedup vs baseline: 1.0119x; 1.0119x over previous
"""RWKV time-mixing (C=4096) on 8 trn2 NeuronCores.

Strategy (tensor-parallel over channels, M=8 cores, S=C/M=512):
  - Core c owns channels sl = [c*512, (c+1)*512).
  - HBM-bound: streams the four weight matrices once in bf16 (16 MB/core;
    rel-err 3.3e-3 vs the 2e-2 budget; fp8 measured over budget). A DMA-only
    probe measured the per-core stream limit at ~425 B/ns on ONE sync-HWDGE
    ring; a dual-ring probe split the same 425 between rings — HBM-capped,
    so all weights ride the single sync ring.
  - Stream order wv, kw, rw, ow: vv lands first, the whole WKV recurrence +
    state update (kk-stage, ~3 us of vector/scalar work) hides under the rw
    stream, and after rw only the short r=sigmoid(rr) -> y=r*wkv -> bf16
    chain (~1.3 us) gates phase 2 — which then chases the arriving ow
    chunks with no other dependency.
  - Every chunk gets a dedicated SBUF buffer (16 MB resident, no reuse
    waits — a reuse stall cost the old kernel ~0.7 us mid-stream).
  - Phase 1: kk/vv/rr shards via host-transposed weights; contraction on
    SBUF partitions, x-vector column stationary, weight tiles moving at
    N=512. The [1,512]->[128,4] transpose of each result runs on the PE
    (zero-padded stationary + e0 unit vector, fp32r).
  - Phase 2: partial = ow[:, sl] @ y as 4 k-tiles x 8 n-banks into a
    [1,4096] PSUM row. ow's last k-tile streams as a 512 KB half (banks
    0-3) plus two 256 KB quarters, so only two stop-matmuls + two parallel
    PSUM->SBUF copies + a 2 KB DMA follow the final weight byte. Copies
    alternate vector/scalar per bank; the output leaves in bf16 as three
    pieces on the sync ring as soon as their banks are staged. Host sums
    the 8 per-core partials in f32.
  - new_state = x exactly (token shift), assembled on host. nst (state
    a/b/p) leaves mid-kernel via gpsimd right after the kk-stage.
  - PE p-state: the PE stalls ~6-8 us after its FIRST matmul, then runs
    ~3 us at half clock. A tiny f32 matmul (dep: one vector memset) fires
    at ~6.3 us so the stall drains while the first chunks stream; pinned
    filler matmuls (deps: last rw chunk / ofull[0]) bridge the two ~1.2 us
    PE-idle windows at the rw->phase-2 transition, which otherwise de-ramp
    the PE to half clock for the first 8 phase-2 matmuls.
  - exec_time is measured from the framework's first MEMSET to the end of
    the (fixed, ~6.7 us) whole-semaphore-file teardown; head before the
    first MEMSET is free, everything after the last output byte counts.
    The last-byte -> consumer semaphore visibility latency is ~1.5 us of
    fixed NOC cost (the sem packet lands 6 ns after the data).
  - The device is bimodal run-to-run (~±10%): in the slow mode a
    power/thermal governor clamps DMA to ~340 B/ns after ~20 us of
    sustained streaming AND halves the PE clock mid-burst (observed with
    zero-gap matmul cadence), on pure-DMA probes too; a mid mode keeps the
    stream fast but stretches the DMA-sem visibility latency 1.5->3.2 us.
    Not controllable from the kernel; fast-mode ~57.7-58.9 us, slow-mode
    ~64-66 us. The post-stream tail is ~13.3 us, of which ~6.9 us is the
    fixed semaphore-file wipe (133 ns/sem, clock-invariant) and ~5.5 us is
    serial sem/NOC/descriptor latencies with no remaining mechanism to
    overlap.

k-index convention (phase 1): k = p*32 + n (p = partition, n = k-tile id),
so W.T.reshape(128, 32, 512) puts k-tile n at [:, n, :] with contraction on
partitions, and x.reshape(128, 32) column n is the matching stationary vec.
WKV-side layout is [128, 4] (channel = j*128 + p), matching the PE-reshape
output and phase-2's per-k-tile contraction over ow[:, sl].T rows.
"""

import ml_dtypes
import numpy as np

import concourse.bass as bass
import concourse.mybir as mybir
import concourse.tile as tile
from concourse import bacc, bass_utils

C = 4096
NCORES = 8
S = C // NCORES          # 512 channels per core
P = 128
KT = C // P              # 32 k-tiles in phase 1
OW_KT = S // P           # 4 k-tiles in phase 2
CHUNKS_FULL = [(0, 8), (8, 8), (16, 8), (24, 8)]

F32 = mybir.dt.float32
F32R = mybir.dt.float32r
BF16 = mybir.dt.bfloat16
AF = mybir.ActivationFunctionType
NP_BF16 = ml_dtypes.bfloat16

# xvecs layout: [128, 96] = xk[0:32] | xv[32:64] | xr[64:96] (bf16)
# svecs layout: [128, 20] = aa[0:4] | bb[4:8] | pp[8:12] | tf[12:16] | td[16:20]
XVECS_W = 3 * KT
SVECS_W = 5 * 4
OFF_XK, OFF_XV, OFF_XR = 0, KT, 2 * KT
OFF_AA, OFF_BB, OFF_PP, OFF_TF, OFF_TD = 0, 4, 8, 12, 16


def _build():
    nc = bacc.Bacc("TRN2", target_bir_lowering=False, debug=False,
                   num_devices=NCORES)

    xvecs_d = nc.dram_tensor("xvecs", [P, XVECS_W], BF16, kind="ExternalInput")
    e0_d = nc.dram_tensor("e0", [P, 2], F32R, kind="ExternalInput")
    svecs_d = nc.dram_tensor("svecs", [P, SVECS_W], F32, kind="ExternalInput")
    wr_d = nc.dram_tensor("wr", [P, KT, S], BF16, kind="ExternalInput")
    wk_d = nc.dram_tensor("wk", [P, KT, S], BF16, kind="ExternalInput")
    wv_d = nc.dram_tensor("wv", [P, KT, S], BF16, kind="ExternalInput")
    wo_d = nc.dram_tensor("wo", [OW_KT, P, C], BF16, kind="ExternalInput")

    # partial leaves in bf16: the per-core quantization (~0.23% rms, random
    # sign) adds ~2e-3 to the 2e-2 budget and halves the critical-path
    # output DMA
    partial_d = nc.dram_tensor("partial", [1, C], BF16, kind="ExternalOutput")
    nst_d = nc.dram_tensor("nst", [P, 12], F32, kind="ExternalOutput")

    with tile.TileContext(nc) as tc:
        with (
            tc.tile_pool(name="const", bufs=1) as const,
            tc.tile_pool(name="w", bufs=12) as wpool,
            tc.tile_pool(name="ow", bufs=3) as opool,
            tc.tile_pool(name="owh", bufs=1) as ohpool,
            tc.tile_pool(name="small", bufs=1) as small,
        ):
            # z2 feeds the PE p-state ramp-trigger matmul; its memset leads
            # the VECTOR queue (vector is otherwise idle until ~20 us) so
            # the trigger fires as early as possible.
            z2 = small.tile([P, 2], F32)
            nc.vector.memset(z2[:], 0.0)

            xvecs = const.tile([P, XVECS_W], BF16)
            e0 = const.tile([P, 2], F32R)
            svecs = const.tile([P, SVECS_W], F32)

            # small inputs on the gpsimd SWDGE ring (the sync ring carries
            # only 8 KB weight rows — a probe showed xvecs' 192 B rows wedge
            # it for ~0.5 us). xvecs first: it gates the first phase-1
            # matmul (~13 us, after the PE p-state stall); e0 is needed by
            # the first PE-reshape (~21 us), svecs by the kk-stage (~30 us).
            nc.gpsimd.dma_start(xvecs[:], xvecs_d[:])
            nc.gpsimd.dma_start(e0[:], e0_d[:])
            nc.gpsimd.dma_start(svecs[:], svecs_d[:])

            # preload the ACT exp LUT off the critical path (the only ACT
            # table the kernel uses: sigmoid is computed via exp+reciprocal)
            warm = small.tile([1, 4], F32)
            nc.gpsimd.memset(warm[:], 0.0)
            warm2 = small.tile([1, 4], F32)
            nc.scalar.activation(warm2[:], warm[:], AF.Exp)

            # stage for the PE reshape: partition 0 carries the phase-1
            # results, rows 1-127 must be finite (they multiply e0's zeros).
            # f32r memset is invalid ISA, so zero an f32 twin and cast-copy
            # (vector; off the critical path).
            stage_z = small.tile([P, 3 * S], F32)
            nc.vector.memset(stage_z[:], 0.0)
            stage = small.tile([P, 3 * S], F32R)
            nc.vector.tensor_copy(stage[:], stage_z[:])

            aa = svecs[:, OFF_AA:OFF_AA + 4]
            bb = svecs[:, OFF_BB:OFF_BB + 4]
            pp = svecs[:, OFF_PP:OFF_PP + 4]
            tf = svecs[:, OFF_TF:OFF_TF + 4]
            td = svecs[:, OFF_TD:OFF_TD + 4]

            def t4(name):
                return small.tile([P, 4], F32, name=name)

            nst = small.tile([P, 12], F32)
            na, nb_t, p2 = nst[:, 0:4], nst[:, 4:8], nst[:, 8:12]
            # rkv layout follows stream order: vv | kk | rr
            rkv = small.tile([P, 12], F32)
            vv = rkv[:, 0:4]
            kk = rkv[:, 4:8]
            rr128 = rkv[:, 8:12]

            wkv = t4("wkv")
            y = t4("y")
            y_r = small.tile([P, 4], BF16, name="y_r")

            # ---- phase 1 + overlapped recurrence ---------------------------
            wdrams = [wv_d, wk_d, wr_d]
            xoffs = [OFF_XV, OFF_XK, OFF_XR]

            with tc.tile_pool(name="ps1", bufs=1, space="PSUM") as ps1:
                psums = [ps1.tile([1, S], F32, name=f"ps_{i}") for i in range(3)]
                rs_ps = ps1.tile([P, 24], F32, name="rs_ps")
                fl_ps = ps1.tile([1, S], F32, name="fl_ps")

                # PE p-state ramp trigger (see module docstring)
                zps = ps1.tile([2, 2], F32, name="zps")
                nc.tensor.matmul(zps[:], lhsT=z2[:], rhs=z2[:], start=True, stop=True)

                last_rw_chunk = None
                for wi in range(3):
                    for ci, (kt0, nkt) in enumerate(CHUNKS_FULL):
                        wt = wpool.tile([P, nkt, S], BF16, tag="wchunk")
                        nc.sync.dma_start(
                            wt[:], wdrams[wi][:, kt0:kt0 + nkt, :])
                        if wi == 2 and ci == len(CHUNKS_FULL) - 1:
                            last_rw_chunk = wt
                        for tl in range(nkt):
                            kt = kt0 + tl
                            nc.tensor.matmul(
                                psums[wi][:],
                                lhsT=xvecs[:, xoffs[wi] + kt:xoffs[wi] + kt + 1],
                                rhs=wt[:, tl, :],
                                start=(kt == 0),
                                stop=(kt == KT - 1),
                            )

                    if wi == 2:
                        # keep the PE busy through the rr CAST window so it
                        # does not de-ramp before phase 2 (measured: an idle
                        # stretch over ~1.2 us halves the PE clock for the
                        # next ~3 us). Deps on the last rw chunk pin these
                        # AFTER the rw matmuls in the schedule.
                        for i in range(3):
                            nc.tensor.matmul(
                                fl_ps[:],
                                lhsT=xvecs[:, i:i + 1],
                                rhs=last_rw_chunk[:, 7, :],
                                start=True,
                                stop=True,
                            )

                    # matrix wi fully reduced: transpose its [1,512] row into
                    # the [128,4] WKV layout while the next matrix streams
                    if wi < 2:
                        nc.vector.tensor_copy(
                            stage[0:1, wi * S:(wi + 1) * S], psums[wi][:])
                    else:
                        nc.scalar.copy(stage[0:1, wi * S:(wi + 1) * S], psums[wi][:])
                    for j in range(OW_KT):
                        c2 = 2 * (wi * 4 + j)
                        nc.tensor.matmul(
                            rs_ps[:, c2:c2 + 2],
                            lhsT=stage[:, wi * S + j * P:wi * S + (j + 1) * P],
                            rhs=e0[:],
                            start=True,
                            stop=True,
                        )
                    nc.vector.tensor_copy(
                        rkv[:, wi * 4:wi * 4 + 4],
                        rs_ps[:, 2 * wi * 4:2 * wi * 4 + 8:2])

                    if wi == 1:
                        # kk-stage: vv and kk are both in hand — compute the
                        # ENTIRE WKV recurrence and state update here; it all
                        # hides under the rw stream.
                        ww1 = t4("ww1")
                        nc.vector.tensor_add(ww1, tf, kk)
                        p1 = t4("p1")
                        nc.vector.tensor_max(p1, pp, ww1)
                        d1 = t4("d1")
                        nc.vector.tensor_sub(d1, pp, p1)
                        e1a = t4("e1a")
                        nc.scalar.activation(e1a, d1, AF.Exp)
                        d2 = t4("d2")
                        nc.vector.tensor_sub(d2, ww1, p1)
                        e2a = t4("e2a")
                        nc.scalar.activation(e2a[:], d2, AF.Exp)
                        acc_a = t4("acc_a")
                        nc.vector.tensor_mul(acc_a[:], e1a, aa)
                        tmp_a = t4("tmp_a")
                        nc.vector.tensor_mul(tmp_a, e2a[:], vv)
                        nc.vector.tensor_add(acc_a[:], acc_a[:], tmp_a)
                        acc_b = t4("acc_b")
                        nc.vector.tensor_mul(acc_b, e1a, bb)
                        nc.vector.tensor_add(acc_b, acc_b, e2a[:])
                        binv = t4("binv")
                        nc.vector.reciprocal(binv[:], acc_b)
                        nc.vector.tensor_mul(wkv[:], acc_a[:], binv[:])
                        # state update (decayed accumulators) -> nst
                        ww2 = t4("ww2")
                        nc.vector.tensor_add(ww2, pp, td)
                        nc.vector.tensor_max(p2, ww2, kk)
                        d3 = t4("d3")
                        nc.vector.tensor_sub(d3, ww2, p2)
                        e1b = t4("e1b")
                        nc.scalar.activation(e1b, d3, AF.Exp)
                        d4 = t4("d4")
                        nc.vector.tensor_sub(d4, kk, p2)
                        e2b = t4("e2b")
                        nc.scalar.activation(e2b[:], d4, AF.Exp)
                        nc.vector.tensor_mul(na, e1b, aa)
                        tmp_b = t4("tmp_b")
                        nc.vector.tensor_mul(tmp_b, e2b[:], vv)
                        nc.vector.tensor_add(na, na, tmp_b)
                        nc.vector.tensor_mul(nb_t, e1b, bb)
                        nc.vector.tensor_add(nb_t, nb_t, e2b[:])
                        # state leaves mid-kernel, fully hidden
                        nc.gpsimd.dma_start(nst_d[:], nst[:])
                    elif wi == 2:
                        # minimal rr tail on the y-critical path:
                        # r = sigmoid(rr) = 1/(1+exp(-rr)), y = r*wkv
                        er = t4("er")
                        nc.scalar.activation(er[:], rr128, AF.Exp, scale=-1.0)
                        rp1 = t4("rp1")
                        nc.vector.tensor_scalar_add(rp1[:], er[:], 1.0)
                        r128 = t4("r128")
                        nc.vector.reciprocal(r128[:], rp1[:])
                        nc.vector.tensor_mul(y[:], r128[:], wkv[:])
                        nc.vector.tensor_copy(y_r[:], y[:])

                # ow weight DMAs: the in-order sync ring plays them right
                # after the phase-1 weight DMAs; by the time the first chunk
                # lands y_r is (nearly) ready and the phase-2 matmuls chase
                # the arriving chunks. k-tiles 0-2 stream as full 1 MB chunks
                # (8 KB rows); k-tile 3 as 2x512KB halves.
                ofull = []
                for tt in range(OW_KT - 1):
                    ot = opool.tile([P, C], BF16, tag="owfull")
                    nc.sync.dma_start(ot[:], wo_d[tt][:, :])
                    ofull.append(ot)
                # tt=3 streams last: banks 0-3 as one 512 KB half, banks 4-5
                # and 6-7 as 256 KB quarters, so the after-last-byte tail is
                # just two N=512 stop-matmuls + two parallel copies + a 4 KB
                # output DMA.
                opieces = []
                for (c0, c1) in ((0, 2048), (2048, 3072), (3072, 4096)):
                    ot = ohpool.tile([P, c1 - c0], BF16, tag=f"owp{c0}")
                    nc.sync.dma_start(ot[:], wo_d[OW_KT - 1][:, c0:c1])
                    opieces.append((c0, ot))

                # bridge the PE from the rr reshape to the first phase-2
                # matmul (the y_r vector chain takes ~1.5 us); deps on
                # ofull[0] pin these after the reshape in the schedule.
                for i in range(3):
                    nc.tensor.matmul(
                        fl_ps[:],
                        lhsT=xvecs[:, 3 + i:4 + i],
                        rhs=ofull[0][:, 0:S],
                        start=True,
                        stop=True,
                    )

            # ---- phase 2: partial = ow[:, sl] @ y ---------------------------
            with tc.tile_pool(name="ps2", bufs=1, space="PSUM") as ps2:
                ow_ps = ps2.tile([1, C], F32)
                out_sb = small.tile([1, C], BF16)
                for tt in range(OW_KT):
                    for nb in range(C // 512):
                        if tt < OW_KT - 1:
                            rhs = ofull[tt][:, nb * 512:(nb + 1) * 512]
                        else:
                            pi = 0 if nb < 4 else (1 if nb < 6 else 2)
                            c0, ot = opieces[pi]
                            rhs = ot[:, nb * 512 - c0:(nb + 1) * 512 - c0]
                        nc.tensor.matmul(
                            ow_ps[:, nb * 512:(nb + 1) * 512],
                            lhsT=y_r[:, tt:tt + 1],
                            rhs=rhs,
                            start=(tt == 0),
                            stop=(tt == OW_KT - 1),
                        )
                        if tt == OW_KT - 1:
                            # bank nb complete; copy out while later banks
                            # are still accumulating (gpsimd cannot read
                            # PSUM, so alternate vector/scalar)
                            sl_ = slice(nb * 512, (nb + 1) * 512)
                            if nb % 2 == 0:
                                nc.vector.tensor_copy(
                                    out_sb[:, sl_], ow_ps[:, sl_])
                            else:
                                nc.scalar.copy(out_sb[:, sl_], ow_ps[:, sl_])
                            if nb in (3, 5, 7):
                                # each output piece leaves as soon as its
                                # banks are staged; only the last 4 KB piece
                                # is on the critical path
                                lo = 0 if nb == 3 else (nb - 1) * 512
                                nc.sync.dma_start(
                                    partial_d[:, lo:(nb + 1) * 512],
                                    out_sb[:, lo:(nb + 1) * 512])

    nc.compile()
    return nc


def _prep_in_maps(x, state, state_a, state_b, state_p,
                  time_mix_k, time_mix_v, time_mix_r,
                  time_first, time_decay, kw, vw, rw, ow):
    f = lambda a: np.ascontiguousarray(np.asarray(a), dtype=np.float32)
    x, state = f(x), f(state)
    tmk, tmv, tmr = f(time_mix_k), f(time_mix_v), f(time_mix_r)
    xk = (x * tmk + state * (1.0 - tmk)).reshape(P, KT)
    xv = (x * tmv + state * (1.0 - tmv)).reshape(P, KT)
    xr = (x * tmr + state * (1.0 - tmr)).reshape(P, KT)
    aa, bb, pp = f(state_a), f(state_b), f(state_p)
    tf, td = f(time_first), f(time_decay)
    kw, vw, rw, ow = f(kw), f(vw), f(rw), f(ow)

    xvecs = np.zeros((P, XVECS_W), dtype=np.float32)
    xvecs[:, OFF_XK:OFF_XK + KT] = xk
    xvecs[:, OFF_XV:OFF_XV + KT] = xv
    xvecs[:, OFF_XR:OFF_XR + KT] = xr
    xvecs = xvecs.astype(NP_BF16)

    e0 = np.zeros((P, 2), dtype=np.float32)
    e0[0, 0] = 1.0

    wb = lambda a: np.ascontiguousarray(a).astype(NP_BF16)

    # WKV-side [128, 4] layout: channel = j*128 + p
    pm = lambda v: np.ascontiguousarray(v.reshape(OW_KT, P).T)
    in_maps = []
    for c in range(NCORES):
        sl = slice(c * S, (c + 1) * S)
        svecs = np.empty((P, SVECS_W), dtype=np.float32)
        svecs[:, OFF_AA:OFF_AA + 4] = pm(aa[sl])
        svecs[:, OFF_BB:OFF_BB + 4] = pm(bb[sl])
        svecs[:, OFF_PP:OFF_PP + 4] = pm(pp[sl])
        svecs[:, OFF_TF:OFF_TF + 4] = pm(tf[sl])
        svecs[:, OFF_TD:OFF_TD + 4] = pm(td[sl])
        in_maps.append({
            "xvecs": xvecs,
            "e0": e0,
            "svecs": svecs,
            "wr": wb(rw[sl, :].T).reshape(P, KT, S),
            "wk": wb(kw[sl, :].T).reshape(P, KT, S),
            "wv": wb(vw[sl, :].T).reshape(P, KT, S),
            "wo": wb(ow[:, sl].T).reshape(OW_KT, P, C),
        })
    return in_maps, x


_NC_CACHE = None


def _run(inputs, trace=False):
    global _NC_CACHE
    if _NC_CACHE is None:
        _NC_CACHE = _build()
    nc = _NC_CACHE
    in_maps, x = _prep_in_maps(**inputs)
    res = bass_utils.run_bass_kernel_spmd(
        nc, in_maps, core_ids=list(range(NCORES)), trace=trace)

    out = np.zeros(C, dtype=np.float32)
    new_a = np.empty(C, dtype=np.float32)
    new_b = np.empty(C, dtype=np.float32)
    new_p = np.empty(C, dtype=np.float32)
    for c in range(NCORES):
        r = res.results[c]
        out += np.asarray(r["partial"], dtype=np.float32).reshape(C)
        sl = slice(c * S, (c + 1) * S)
        nst = r["nst"]
        # [p, j] -> channel j*128 + p
        new_a[sl] = nst[:, 0:4].T.reshape(S)
        new_b[sl] = nst[:, 4:8].T.reshape(S)
        new_p[sl] = nst[:, 8:12].T.reshape(S)
    return (out, x.copy(), new_a, new_b, new_p), res


def kernel(**inputs):
    outs, _ = _run(inputs, trace=False)
    return outs
